# revision 37
# baseline (speedup 1.0000x reference)
"""BatchHardTripletLoss on 8 TRN2 NeuronCores (Bass/Tile).

Wall-clock through the axon tunnel is transfer-dominated (~40 MB/s plus
~100 ms of fixed per-call RPC latency), so the design minimizes bytes on
the wire:

  - Each core ships ONE fp8(e4m3) tensor pack[141, 1024] holding its 1/8 of
    the batch: rows 0..127 = (e/SE)^T, rows 128..137 = one-hot(label), rows
    138..140 = -sq'/(2*SQSC) split into three fp8 limbs (sq' = ||e||^2/SE^2;
    SE is a power-of-2 scale, 1 for standard-normal data, that keeps fp8 in
    range for any input magnitude).  141 KB/core vs 11.1 MB/core for the
    replicated f32 layout.  (End-to-end fp8 loss error: 5.6e-4, ~36x under
    the 2e-2 gate.)
  - On chip, an AllGather (DRAM bounce buffers) rebuilds the full candidate
    matrices on every core.
  - Two accumulating matmuls per (anchor-tile, 512-wide j-chunk) compute
        val2[a, j] = e'_a.e'_j - sq'_j/2 - (BIG/2) * same(a, j)
    contracting: the 128 raw fp8 e rows (used as BOTH stationary and moving
    operand - no on-chip fp8 transform needed), plus a combined 13-row bf16
    aux matmul: rows 0..9 = one-hot (lhs scaled by -BIG/2), rows 10..12 =
    sq limbs against a constant-SQSC lhs (built by SBUF->SBUF DMA, since
    engine ops cannot start at partition 10).  Masked same-class entries
    (~ -16.3k) always sit far below real different-class values, so
        reduce_max_j val2 -> hardest-negative partial
        reduce_min_j val2 -> hardest-positive partial (offset by -BIG/2)
    i.e. ONE value matrix serves both reductions.
  - Output is just [128, 16] f32 per core (hn/hp partials per anchor tile);
    the host rescales by 2*SE^2 and finishes with sqrt/relu/mean in f64:
        hn = sqrt(sq_a - hn_m),  hp = sqrt(sq_a - hp_m),
        loss = mean(relu(hp - hn + 1)).

The other big win is caching: bass2jax's neuronx_cc_hook recompiles the
NEFF on every run_bass_kernel_spmd call (fresh jit closure, no cache on the
bass_exec path).  Enabling JAX's persistent compilation cache short-circuits
the whole compile on warm calls.
"""

import numpy as np
import ml_dtypes

import jax

# The bass_exec path in neuronx_cc_hook has no NEFF cache: every fresh
# jax.jit closure inside run_bass_kernel_spmd re-runs compile_bir_kernel
# (~150ms+ per call).  JAX's persistent compilation cache short-circuits
# that: warm calls load the compiled executable from disk and never reach
# the hook.
jax.config.update("jax_compilation_cache_dir", "/tmp/jax_comp_cache")
jax.config.update("jax_persistent_cache_min_compile_time_secs", 0.0)
jax.config.update("jax_persistent_cache_min_entry_size_bytes", 0)

import concourse.bass as bass
import concourse.bacc as bacc
import concourse.bass2jax as bass2jax
import concourse.tile as tile
from concourse import mybir
from concourse.bass_utils import run_bass_kernel_spmd

B, D, NCLASS = 8192, 128, 10
NCORES = 8
S = B // NCORES            # 1024 rows per core
NAUX = 13                  # aux rows: 10 one-hot + 3 sq limbs
PACKP = D + NAUX           # 141
NAT = S // 128             # 8 anchor tiles per core
JC = 512                   # j-chunk width (one PSUM bank)
NJC = B // JC              # 16 chunks
BIG = 32768.0              # exact in bf16 (2^15)
SQSC = 8.0                 # sq-limb scale: limbs hold -sq'/(2*SQSC), the
                           # lhs constant rows hold SQSC, so fp8 limbs stay
                           # within e4m3 range (240) up to sq' ~ 3840
MARGIN = 1.0
F32 = mybir.dt.float32
BF16 = mybir.dt.bfloat16
FP8 = mybir.dt.float8e4
ALU = mybir.AluOpType
BFNP = np.dtype(ml_dtypes.bfloat16)
F8NP = np.dtype(ml_dtypes.float8_e4m3)

_NC_CACHE = None

# ---------------------------------------------------------------------------
# run_bass_via_pjrt rebuilds its jit closure (trace + lower + compile-cache
# read + executable rehydrate, ~13ms) on every call even though nothing about
# nc changes between calls.  Memoize the jitted callable per nc, keeping the
# run_bass_kernel_spmd -> run_bass_via_pjrt call chain and semantics intact
# (fresh numpy inputs are transferred and donated per call exactly as
# before).  Unknown configurations fall back to the original.
_orig_run_bass_via_pjrt = bass2jax.run_bass_via_pjrt
_RBVP_CACHE = {}


def _cached_run_bass_via_pjrt(nc, in_maps, n_cores):
    if n_cores <= 1 or nc.dbg_addr is not None:
        return _orig_run_bass_via_pjrt(nc, in_maps, n_cores=n_cores)
    ent = _RBVP_CACHE.get(id(nc))
    if ent is None or ent["nc"] is not nc or ent["n_cores"] != n_cores:
        from jax.sharding import Mesh, PartitionSpec
        from jax.experimental.shard_map import shard_map

        bass2jax.install_neuronx_cc_hook()
        partition_name = (nc.partition_id_tensor.name
                          if nc.partition_id_tensor else None)
        in_names, out_names, out_avals = [], [], []
        for alloc in nc.m.functions[0].allocations:
            if not isinstance(alloc, mybir.MemoryLocationSet):
                continue
            name = alloc.memorylocations[0].name
            if alloc.kind == "ExternalInput":
                if name != partition_name:
                    in_names.append(name)
            elif alloc.kind == "ExternalOutput":
                shape = tuple(alloc.tensor_shape)
                dtype = mybir.dt.np(alloc.dtype)
                out_names.append(name)
                out_avals.append(jax.core.ShapedArray(shape, dtype))
        # No donated zero-init output buffers: this kernel writes every
        # output element, so results can come from fresh (uninit) XLA
        # allocations and the zeros upload is dropped entirely.
        in_names_all = list(in_names)
        if partition_name is not None:
            in_names_all.append(partition_name)

        def _body(*args):
            operands = list(args)
            if partition_name is not None:
                operands.append(bass2jax.partition_id_tensor())
            outs = bass2jax._bass_exec_p.bind(
                *operands, out_avals=tuple(out_avals),
                in_names=tuple(in_names_all), out_names=tuple(out_names),
                lowering_input_output_aliases=(),
                sim_require_finite=True, sim_require_nnan=True, nc=nc)
            return tuple(outs)

        devices = jax.devices()[:n_cores]
        assert len(devices) == n_cores
        mesh = Mesh(np.asarray(devices), ("core",))
        # When the kernel AllGathers its outputs on chip (every core holds
        # the identical full result), declare the jit outputs replicated:
        # jax then fetches a single shard instead of one per core.
        replicated = bool(getattr(nc, "_replicated_outputs", False))
        out_spec = PartitionSpec() if replicated else PartitionSpec("core")
        sharded = jax.jit(
            shard_map(_body, mesh=mesh,
                      in_specs=(PartitionSpec("core"),) * len(in_names),
                      out_specs=(out_spec,) * len(out_names),
                      check_rep=False),
            keep_unused=True)
        ent = {"nc": nc, "n_cores": n_cores, "sharded": sharded,
               "in_names": in_names, "out_names": out_names,
               "out_avals": out_avals, "replicated": replicated}
        _RBVP_CACHE[id(nc)] = ent

    per_core = [[np.asarray(m[name]) for name in ent["in_names"]]
                for m in in_maps]
    concat_in = [np.concatenate([per_core[c][i] for c in range(n_cores)],
                                axis=0) for i in range(len(ent["in_names"]))]
    out_arrs = ent["sharded"](*concat_in)
    fetched = [np.asarray(a) for a in out_arrs]
    if ent["replicated"]:
        return [{name: fetched[i] for i, name in enumerate(ent["out_names"])}
                for c in range(n_cores)]
    return [
        {name: fetched[i].reshape(n_cores, *ent["out_avals"][i].shape)[c]
         for i, name in enumerate(ent["out_names"])}
        for c in range(n_cores)
    ]


bass2jax.run_bass_via_pjrt = _cached_run_bass_via_pjrt


def build_nc():
    # partition_id is unused (the program is SPMD-uniform; the collective
    # handles placement), so skip its tensor to drop one jit operand.
    nc = bacc.Bacc(num_devices=NCORES, enable_partition_id=False)
    pack_d = nc.dram_tensor("pack", [PACKP, S], FP8, kind="ExternalInput")
    # The per-core [128, 16] partials are AllGathered on chip so every core
    # outputs the identical full result: the jit output can then be declared
    # replicated and the host fetches ONE shard instead of eight.
    res_d = nc.dram_tensor("res", [NCORES * 128, 2 * NAT], F32,
                           kind="ExternalOutput")

    with tile.TileContext(nc) as tc:
        with (
            tc.tile_pool(name="sb", bufs=1) as sb,
            tc.tile_pool(name="dram", bufs=1, space="DRAM") as dram,
            tc.tile_pool(name="psum", bufs=2, space=bass.MemorySpace.PSUM) as psum,
        ):
            # ---- AllGather the packed slice to every core (DRAM bounce) ----
            in_b = dram.tile([PACKP, S], FP8, tag="in_b")
            out_b = dram.tile([NCORES * PACKP, S], FP8, tag="out_b")
            nc.gpsimd.dma_start(in_b[:], pack_d[:])
            nc.gpsimd.collective_compute(
                "AllGather",
                ALU.bypass,
                replica_groups=[list(range(NCORES))],
                ins=[in_b.opt()],
                outs=[out_b.opt()],
            )

            # ---- candidate-side operands (full batch, from the gather) ----
            # Separate tiles per row group so every engine op starts at
            # partition 0 (mid-tile partition offsets fail BIR verification).
            # e stays fp8 (matmul operand only); one-hot and sq rows are
            # widened to bf16 on chip so the mask scale (-BIG/2) fits.
            full_e = sb.tile([D, B], FP8, tag="full_e")
            full_x = sb.tile([NAUX, B], FP8, tag="full_x")
            for c in range(NCORES):
                r = c * PACKP
                nc.sync.dma_start(full_e[:, c * S:(c + 1) * S],
                                  out_b[r:r + D, :])
                nc.sync.dma_start(full_x[:, c * S:(c + 1) * S],
                                  out_b[r + D:r + PACKP, :])
            rx = sb.tile([NAUX, B], BF16, tag="rx")
            nc.vector.tensor_copy(rx[:], full_x[:])

            # ---- anchor-side operands (this core's own slice) ----
            # lhs for the main matmul is the raw fp8 e slice: the kernel
            # computes val2 = e.e - sq/2 - (BIG/2)*same and the host doubles.
            eloc = sb.tile([D, S], FP8, tag="eloc")
            nc.sync.dma_start(eloc[:], pack_d[0:D, :])
            xloc = sb.tile([NAUX, S], FP8, tag="xloc")
            nc.sync.dma_start(xloc[:], pack_d[D:PACKP, :])

            # Combined aux lhs [13, S]: rows 0..9 = -BIG/2 * onehot, rows
            # 10..12 = 1 (to pick up the candidate sq limbs).  Engine ops
            # cannot start at partition 10, but a DMA can, so the ones rows
            # are built at partition 0 and DMA'd into place.
            lhs_x = sb.tile([NAUX, S], BF16, tag="lhs_x")
            nc.vector.tensor_scalar_mul(lhs_x[0:NCLASS, :], xloc[0:NCLASS, :],
                                        -BIG / 2)
            sc3 = sb.tile([3, S], BF16, tag="sc3")
            nc.vector.memset(sc3[:], float(SQSC))
            nc.sync.dma_start(lhs_x[NCLASS:NAUX, :], sc3[:])

            # ---- main loop: 8 anchor tiles x 4 PSUM-quarter chunks ----
            # A [128, 2048] PSUM tile (4 banks, double-buffered) amortizes
            # one reduce_max/reduce_min over four matmul-triples, replacing
            # the per-512-chunk reduces (fewer instructions -> faster
            # per-call lowering, which is on the warm critical path).
            QW = 2048                  # PSUM quarter width
            NQ = B // QW               # 4 quarters
            hn_all = sb.tile([128, NAT * NQ], F32, tag="hn_all")
            hp_all = sb.tile([128, NAT * NQ], F32, tag="hp_all")
            for t in range(NAT):
                a0 = t * 128
                for q in range(NQ):
                    ps = psum.tile([128, QW], F32, tag="ps")
                    for k in range(QW // JC):
                        j0 = q * QW + k * JC
                        p0 = k * JC
                        nc.tensor.matmul(ps[:, p0:p0 + JC],
                                         eloc[:, a0:a0 + 128],
                                         full_e[:, j0:j0 + JC],
                                         start=True, stop=False)
                        nc.tensor.matmul(ps[:, p0:p0 + JC],
                                         lhs_x[:, a0:a0 + 128],
                                         rx[:, j0:j0 + JC],
                                         start=False, stop=True)
                    col = t * NQ + q
                    nc.vector.tensor_reduce(hn_all[:, col:col + 1], ps[:],
                                            axis=mybir.AxisListType.X,
                                            op=ALU.max)
                    nc.vector.tensor_reduce(hp_all[:, col:col + 1], ps[:],
                                            axis=mybir.AxisListType.X,
                                            op=ALU.min)

            # ---- fold quarters, ship [128, 16] ----
            res_sb = sb.tile([128, 2 * NAT], F32, tag="res_sb")
            for t in range(NAT):
                nc.vector.tensor_reduce(res_sb[:, t:t + 1],
                                        hn_all[:, t * NQ:(t + 1) * NQ],
                                        axis=mybir.AxisListType.X, op=ALU.max)
                nc.vector.tensor_reduce(res_sb[:, NAT + t:NAT + t + 1],
                                        hp_all[:, t * NQ:(t + 1) * NQ],
                                        axis=mybir.AxisListType.X, op=ALU.min)
            in_b2 = dram.tile([128, 2 * NAT], F32, tag="in_b2")
            out_b2 = dram.tile([NCORES * 128, 2 * NAT], F32, tag="out_b2")
            nc.gpsimd.dma_start(in_b2[:], res_sb[:])
            nc.gpsimd.collective_compute(
                "AllGather",
                ALU.bypass,
                replica_groups=[list(range(NCORES))],
                ins=[in_b2.opt()],
                outs=[out_b2.opt()],
            )
            nc.gpsimd.dma_start(res_d[:], out_b2[:])
    nc.compile()
    nc._replicated_outputs = True
    return nc


def prepare(embeddings, labels):
    emb = np.ascontiguousarray(np.asarray(embeddings, dtype=np.float32))
    lab = np.asarray(labels).ravel().astype(np.int64)
    assert emb.shape == (B, D) and lab.shape == (B,)
    sq = np.sum(emb.astype(np.float64) ** 2, axis=1)          # [B] f64
    oh = (lab[None, :] == np.arange(NCLASS)[:, None])         # [10, B]
    # Power-of-2 embedding scale keeps sq' = sq/SE^2 <= ~3840 so the fp8
    # limbs (-sq'/(2*SQSC), |.| <= 240) and the -BIG/2 mask margin hold for
    # any input magnitude.  SE == 1 for standard-normal data (no effect).
    max_sq = float(sq.max()) if B else 1.0
    se = 1.0
    while max_sq / (se * se) > 3840.0:
        se *= 2.0
    h = -sq / (se * se * 2 * SQSC)                            # 3 fp8 limbs
    m1 = h.astype(F8NP)
    r = h - m1.astype(np.float64)
    m2 = r.astype(F8NP)
    m3 = (r - m2.astype(np.float64)).astype(F8NP)
    es = (emb / se).astype(F8NP)
    in_maps = []
    for i in range(NCORES):
        sl = slice(i * S, (i + 1) * S)
        pack = np.empty((PACKP, S), dtype=F8NP)
        pack[0:D] = es[sl].T
        pack[D:D + NCLASS] = oh[:, sl].astype(F8NP)
        pack[D + NCLASS] = m1[sl]
        pack[D + NCLASS + 1] = m2[sl]
        pack[D + NCLASS + 2] = m3[sl]
        in_maps.append({"pack": pack})
    return in_maps, (sq, se)


def combine(results, sq_se):
    sq, se = sq_se
    se2 = se * se
    total = 0.0
    for i in range(NCORES):
        # Each core outputs the full gathered [8*128, 16]; take core i's block.
        res = np.asarray(results[i]["res"],
                         np.float32)[i * 128:(i + 1) * 128].astype(np.float64)
        # res[p, t] covers local anchor t*128+p -> transpose to local order.
        # The kernel computed val2' = (2e.e - sq)/(2*SE^2) - (BIG/2)*same:
        # scale back by 2*SE^2 and undo the -BIG*same offset on the min.
        hn_m = 2.0 * se2 * res[:, 0:NAT].T.reshape(S)
        hp_m = se2 * (2.0 * res[:, NAT:2 * NAT].T.reshape(S) + BIG)
        sq_a = sq[i * S:(i + 1) * S]
        hn = np.sqrt(np.maximum(sq_a - hn_m, 0.0))
        hp = np.sqrt(np.maximum(sq_a - hp_m, 0.0))
        total += float(np.sum(np.maximum(hp - hn + MARGIN, 0.0)))
    return np.asarray(total / B, dtype=np.float32)


def kernel(embeddings, labels):
    global _NC_CACHE
    in_maps, sq = prepare(embeddings, labels)
    if _NC_CACHE is None:
        _NC_CACHE = build_nc()
    res = run_bass_kernel_spmd(_NC_CACHE, in_maps, list(range(NCORES)))
    return combine(res.results, sq)
